# revision 1
# baseline (speedup 1.0000x reference)
"""Trainium2 Bass kernel for vector-neuron multi-head attention.

Full-input contract: kernel(**inputs) takes the unsharded inputs and
returns the full [4, 256, 3, 2048] output.

Sharding: 8 cores = 4 batches x 2 query-halves (m-split). Each core
computes projections + attention for ALL 8 heads of one batch, but only
for its 1024 of the 2048 queries, producing the final projected output
slice [256, 3, 1024]. No collectives; host concatenates slices.

Per-core pipeline (matmul operands fp16, accumulation fp32):
  - q/k/z projections done per e-half (128 output channels at a time) to
    halve SBUF pressure; bias u = EPS*b/||b|| (host-precomputed, fp32) is
    added in fp32 during PSUM eviction before the fp16 rounding.
  - per-head packed tiles qf [96, M], kf [96, N], zf [97, N] (row 96 of
    zf is ones) built with SBUF->SBUF DMA.
  - scores computed TRANSPOSED: st[n, m] = kf-slice.T @ qf, so softmax
    needs no max pass (scores are O(1), exp is safe in fp32) and A never
    needs transposing for the AV matmul.
  - exp on ScalarE reads fp32 PSUM scores with the softmax scale fused
    into the activation, writes fp16.
  - AV accumulates zfT_aug.T @ exp(st) over n-tiles in fp32 PSUM; the
    ones row of zf makes row 96 of the result the softmax denominator.
  - normalize in fp32: DVE reciprocal of row 96, PE-broadcast (fp32
    matmul against a ones column), multiply; head outputs repacked
    c-major via DMA.
  - final Wo projection with fp32 bias add fused into eviction.
"""

from contextlib import ExitStack

import numpy as np

import concourse.bacc as bacc
import concourse.bass as bass
import concourse.tile as tile
from concourse import mybir
from concourse.bass_utils import run_bass_kernel_spmd

FP32 = mybir.dt.float32
BF16 = mybir.dt.float16  # fp16: 10 mantissa bits, same PE speed as bf16
AF = mybir.ActivationFunctionType
ALU = mybir.AluOpType

EMB = 256
HEADS = 8
EPS = 1e-6
B = 4
N = 2048          # key/value length
ML = 1024         # queries per core (m-half)
CH = 32           # channels per head
SCALE = 1.0 / np.sqrt(3.0 * CH)
NT = N // 128     # 16 n-tiles
P = 128


def ts(i, s):
    return slice(i * s, (i + 1) * s)


def build_nc(nrep=1):
    nc = bacc.Bacc("TRN2", target_bir_lowering=False, debug=False)

    xq = nc.dram_tensor("xq", [EMB, 3, ML], BF16, kind="ExternalInput").ap()
    xk = nc.dram_tensor("xk", [EMB, 3, N], BF16, kind="ExternalInput").ap()
    xz = nc.dram_tensor("xz", [EMB, 3, N], BF16, kind="ExternalInput").ap()
    ws = {
        t: nc.dram_tensor(f"w{t}", [EMB, EMB], BF16, kind="ExternalInput").ap()
        for t in ("q", "k", "z", "o")
    }
    us = {
        t: nc.dram_tensor(f"u{t}", [EMB, 3], FP32, kind="ExternalInput").ap()
        for t in ("q", "k", "z", "o")
    }
    ident = nc.dram_tensor("ident", [P, P], BF16, kind="ExternalInput").ap()
    y = nc.dram_tensor("y", [EMB, 3, ML], FP32, kind="ExternalOutput").ap()

    # DRAM views: channel dim split into (chunk, partition)
    xr = {
        "q": xq.rearrange("(c p) d t -> p c d t", p=P),
        "k": xk.rearrange("(c p) d t -> p c d t", p=P),
        "z": xz.rearrange("(c p) d t -> p c d t", p=P),
    }
    wr = {t: w.rearrange("(c p) e -> p c e", p=P) for t, w in ws.items()}
    ur = {t: u.rearrange("(c p) d -> p c d", p=P) for t, u in us.items()}
    yr = y.rearrange("(c p) d t -> p c d t", p=P)

    with tile.TileContext(nc) as tc:
        with ExitStack() as ctx:
            pool = lambda name, bufs, **kw: ctx.enter_context(
                tc.tile_pool(name=name, bufs=bufs, **kw)
            )
            consts = pool("consts", 1)
            xin_pool = pool("xin", 4)
            qproj_pool = pool("qproj", 2)
            kproj_pool = pool("kproj", 2)
            zproj_pool = pool("zproj", 2)
            qf_pool = pool("qf", 3)
            kf_pool = pool("kf", 3)
            zf_pool = pool("zf", 3)
            zft_pool = pool("zft", 2)
            expst_pool = pool("expst", 4)
            inv_pool = pool("inv", 2)
            invb_pool = pool("invb", 2)
            outh_pool = pool("outh", 2)
            outall_pool = pool("outall", 2)
            yp_pool = pool("ypiece", 4)
            pst_pool = pool("pst", 2, space="PSUM")
            pav_pool = pool("pav", 1, space="PSUM")
            pzt_pool = pool("pzt", 1, space="PSUM")
            pproj_pool = pool("pproj", 1, space="PSUM")

            # constants
            w_sb = {}
            u_sb = {}
            for t in ("q", "k", "z", "o"):
                w_sb[t] = consts.tile([P, 2, EMB], BF16, tag=f"w{t}", name=f"w{t}_sb")
                nc.sync.dma_start(out=w_sb[t], in_=wr[t])
                u_sb[t] = consts.tile([P, 2, 3], FP32, tag=f"u{t}", name=f"u{t}_sb")
                nc.sync.dma_start(out=u_sb[t], in_=ur[t])
            ident_sb = consts.tile([P, P], BF16, tag="ident")
            nc.sync.dma_start(out=ident_sb, in_=ident)
            ones96 = consts.tile([1, 96], BF16, tag="ones96")
            nc.vector.memset(ones96, 1.0)

            # body below may be emitted nrep times (timing builds measure
            # the marginal per-rep cost; nrep=1 for normal use)
            for rep in range(nrep):
                out_all = [
                    outall_pool.tile([P, 3, ML], BF16, tag="outall", name=f"outall{i}")
                    for i in range(2)
                ]
                all_projs = [{}, {}]

                def proj_work(half, drip):
                    """One (tensor, d, nt) projection piece per next(). Half-1
                    pieces are drip-fed into half-0's attention loop and use a
                    dedicated PSUM bank to stay out of the scores pipeline."""
                    for t, T, ppool in (
                        ("q", ML, qproj_pool),
                        ("k", N, kproj_pool),
                        ("z", N, zproj_pool),
                    ):
                        proj = ppool.tile(
                            [P, 3, T], BF16, tag=f"{t}proj", name=f"{t}proj{half}"
                        )
                        all_projs[half][t] = proj
                        for d in range(3):
                            xin = xin_pool.tile(
                                [P, 2, 2048], BF16, tag="xin", name="xin"
                            )
                            for ch in range(T // 512):
                                nc.sync.dma_start(
                                    out=xin[:, :, ts(ch, 512)],
                                    in_=xr[t][:, :, d, ts(ch, 512)],
                                )
                            for nt in range(T // 512):
                                pool_ = pproj_pool if drip else pst_pool
                                ps = pool_.tile(
                                    [P, 512],
                                    FP32,
                                    tag="pproj" if drip else "pst",
                                    name="projps",
                                )
                                for cc in range(2):
                                    nc.tensor.matmul(
                                        ps,
                                        lhsT=w_sb[t][:, cc, ts(half, P)],
                                        rhs=xin[:, cc, ts(nt, 512)],
                                        start=(cc == 0),
                                        stop=(cc == 1),
                                    )
                                # fp32 bias add + fp16 cast on eviction
                                nc.vector.tensor_scalar_add(
                                    proj[:, d, ts(nt, 512)],
                                    ps,
                                    u_sb[t][:, half, d : d + 1],
                                )
                                yield

                gens = [proj_work(0, drip=False), proj_work(1, drip=True)]
                for _ in gens[0]:  # half-0 projections up front
                    pass

                def repack(h):
                    """Issue the per-head repack DMAs (qf/kf/zf) one head
                    ahead so they land before the PE needs the tiles."""
                    half, j = divmod(h, 4)
                    r0 = 32 * j
                    projs = all_projs[half]
                    qf = qf_pool.tile([96, ML], BF16, tag="qf", name=f"qf{h}")
                    kf = kf_pool.tile([96, N], BF16, tag="kf", name=f"kf{h}")
                    zf = zf_pool.tile([97, N], BF16, tag="zf", name=f"zf{h}")
                    for d in range(3):
                        nc.sync.dma_start(
                            out=qf[ts(d, 32), :], in_=projs["q"][r0 : r0 + 32, d, :]
                        )
                        nc.sync.dma_start(
                            out=kf[ts(d, 32), :], in_=projs["k"][r0 : r0 + 32, d, :]
                        )
                        nc.sync.dma_start(
                            out=zf[ts(d, 32), :], in_=projs["z"][r0 : r0 + 32, d, :]
                        )
                    nc.gpsimd.memset(zf[96:97, :], 1.0)
                    return qf, kf, zf

                def transposes(h, zf):
                    """zf -> 16 [128, 97] lhsT tiles (4 per psum tile);
                    emitted mid-previous-head so the PE work hides under the
                    ACT exp backlog."""
                    zfts = zft_pool.tile([P, NT, 98], BF16, tag="zft", name=f"zfts{h}")
                    for g in range(4):
                        zt = pzt_pool.tile([P, 4, 98], BF16, tag="pzt", name="zt")
                        for jj in range(4):
                            nt = 4 * g + jj
                            nc.tensor.transpose(
                                zt[:, jj, :97],
                                zf[:, ts(nt, P)],
                                ident_sb[:97, :97],
                            )
                        nc.vector.tensor_copy(zfts[:, ts(g, 4), :97], zt[:, :, :97])
                    return zfts

                tiles = {0: repack(0)}
                for h in range(8):
                    if h + 1 < 8:
                        tiles[h + 1] = repack(h + 1)
                    qf, kf, zf_h = tiles.pop(h)
                    zfts = transposes(h, zf_h)
                    half = h // 4
                    r0 = 32 * (h % 4)

                    av = pav_pool.tile([97, ML], FP32, tag="pav", name="av")
                    for nt in range(NT):
                        st = pst_pool.tile([P, ML], FP32, tag="pst", name="st")
                        for mc in range(ML // 512):
                            nc.tensor.matmul(
                                st[:, ts(mc, 512)],
                                lhsT=kf[:, ts(nt, P)],
                                rhs=qf[:, ts(mc, 512)],
                                start=True,
                                stop=True,
                            )
                        ex = expst_pool.tile([P, ML], BF16, tag="expst", name="ex")
                        nc.scalar.activation(ex, st, AF.Exp, scale=float(SCALE))
                        for mc in range(ML // 512):
                            nc.tensor.matmul(
                                av[:, ts(mc, 512)],
                                lhsT=zfts[:, nt, :97],
                                rhs=ex[:, ts(mc, 512)],
                                start=(nt == 0),
                                stop=(nt == NT - 1),
                            )
                        if h < 3 and (nt * 5) % 8 < 5:
                            # drip-feed ~0.625 half-1 projection pieces per
                            # n-tile so PE stays just under ACT's exp rate
                            next(gens[1], None)

                    # evict av to SBUF right away so its PSUM slot frees for
                    # the next head; row 96 is the softmax denominator
                    av_sb = invb_pool.tile([97, ML], FP32, tag="avsb", name="av_sb")
                    nc.vector.tensor_copy(av_sb, av)
                    inv = inv_pool.tile([1, ML], BF16, tag="inv", name="inv")
                    with nc.allow_low_precision(reason="softmax inverse in fp16"):
                        nc.vector.reciprocal(inv, av_sb[96:97, :])
                    outh = outh_pool.tile([96, ML], BF16, tag="outh", name="outh")
                    for mc in range(ML // 512):
                        invb_ps = pproj_pool.tile(
                            [96, 512], FP32, tag="pproj", name="invb_ps"
                        )
                        nc.tensor.matmul(
                            invb_ps,
                            lhsT=ones96,
                            rhs=inv[:, ts(mc, 512)],
                            start=True,
                            stop=True,
                        )
                        nc.vector.tensor_tensor(
                            outh[:, ts(mc, 512)],
                            av_sb[0:96, ts(mc, 512)],
                            invb_ps,
                            ALU.mult,
                        )
                    for d in range(3):
                        nc.sync.dma_start(
                            out=out_all[half][r0 : r0 + 32, d, :],
                            in_=outh[ts(d, 32), :],
                        )

                for _ in gens[1]:  # safety flush (normally exhausted)
                    pass

                # ---- final projection ----
                for eo in range(2):
                    for d in range(3):
                        for mt in range(ML // 512):
                            ps = pst_pool.tile([P, 512], FP32, tag="pst", name="yps")
                            for cc in range(2):
                                nc.tensor.matmul(
                                    ps,
                                    lhsT=w_sb["o"][:, cc, ts(eo, P)],
                                    rhs=out_all[cc][:, d, ts(mt, 512)],
                                    start=(cc == 0),
                                    stop=(cc == 1),
                                )
                            yp = yp_pool.tile([P, 512], FP32, tag="ypiece", name="yp")
                            nc.scalar.activation(
                                yp, ps, AF.Identity, bias=u_sb["o"][:, eo, d : d + 1]
                            )
                            nc.sync.dma_start(out=yr[:, eo, d, ts(mt, 512)], in_=yp)

    nc.compile()
    return nc


_NC_CACHE = {}


def get_nc():
    if "nc" not in _NC_CACHE:
        _NC_CACHE["nc"] = build_nc()
    return _NC_CACHE["nc"]


def make_in_maps(Q, K, Z, Wq_w, Wq_b, Wk_w, Wk_b, Wz_w, Wz_b, Wo_w, Wo_b):
    bf16 = mybir.dt.np(BF16)

    def u_of(b):
        b = np.asarray(b, np.float32)
        return (EPS * b / np.linalg.norm(b, axis=1, keepdims=True)).astype(np.float32)

    common = {
        "wq": np.ascontiguousarray(Wq_w).astype(bf16),
        "wk": np.ascontiguousarray(Wk_w).astype(bf16),
        "wz": np.ascontiguousarray(Wz_w).astype(bf16),
        "wo": np.ascontiguousarray(Wo_w).astype(bf16),
        "uq": u_of(Wq_b),
        "uk": u_of(Wk_b),
        "uz": u_of(Wz_b),
        "uo": u_of(Wo_b),
        "ident": np.eye(P, dtype=np.float32).astype(bf16),
    }
    Qb = np.asarray(Q).astype(bf16)
    Kb = np.asarray(K).astype(bf16)
    Zb = np.asarray(Z).astype(bf16)
    in_maps = []
    for core in range(8):
        b, mh = core // 2, core % 2
        in_maps.append(
            dict(
                common,
                xq=np.ascontiguousarray(Qb[b][:, :, mh * ML : (mh + 1) * ML]),
                xk=np.ascontiguousarray(Kb[b]),
                xz=np.ascontiguousarray(Zb[b]),
            )
        )
    return in_maps


def assemble(results):
    out = np.empty((B, EMB, 3, N), dtype=np.float32)
    for core in range(8):
        b, mh = core // 2, core % 2
        out[b][:, :, mh * ML : (mh + 1) * ML] = results[core]["y"]
    return out


def kernel(**inputs):
    nc = get_nc()
    in_maps = make_in_maps(**inputs)
    res = run_bass_kernel_spmd(nc, in_maps, list(range(8)))
    return assemble(res.results)


if __name__ == "__main__":
    nc = build_nc()
    print("built ok")



# revision 5
# speedup vs baseline: 2.8267x; 2.8267x over previous
"""Trainium2 Bass kernel for vector-neuron multi-head attention, v2.2.

Sharding: 8 cores = 4 batches x 2 head-groups (tensor parallel). Each core
projects q/k/z for its 4 heads (128 of 256 output channels) over the full
M=N=2048 tokens, runs attention for those heads, and computes a PARTIAL
final Wo projection (contraction over its 128 channels only). The host
sums the two partials per batch and adds the Wo bias.

Layout tricks:
  - wq/wk columns are host-PERMUTED (ch-major, head-minor) so the per-head
    (d,ch)-packed qf/kf tiles are natural-partition-order fanout DMAs from
    the projection output ([128, chunk] -> [32, 4h, chunk]).
  - wz keeps the original head-contiguous layout; the AV operand zfts
    [n, (d*32+ch)+ones] is built by PE transposes DIRECTLY from the z
    projection (32-row tiles at base partition 32h via tile_position).
  - Wo rows get the q/k permutation so the reverse gather DMA feeds the
    final matmul directly. y partials are written bf16 (summed on host).

Scheduling: every engine queue is in-order, so cross-step overlap must be
EMITTED interleaved. All non-attention work (next rep's input loads,
projections, fanouts, transposes; this rep's per-m-half final projection)
is chopped into generator pieces and dripped one per odd n-tile slot of
the attention blocks (8 blocks x 16 slots/rep). ACT then runs its 128
softmax exps (~133us/rep) nearly back-to-back, with PE (~136us/rep)
co-bound. ~30 DMAs/rep.
"""

from collections import deque
from contextlib import ExitStack

import numpy as np

import concourse.bacc as bacc
import concourse.bass as bass
import concourse.tile as tile
from concourse import mybir
from concourse.bass_utils import run_bass_kernel_spmd

FP32 = mybir.dt.float32
BF16 = mybir.dt.float16  # fp16: 10 mantissa bits, same PE speed as bf16
AF = mybir.ActivationFunctionType
ALU = mybir.AluOpType

EMB = 256
HEADS = 8
EPS = 1e-6
B = 4
N = 2048          # tokens (M = N here)
HL = 4            # heads per core
CH = 32           # channels per head
SCALE = 1.0 / np.sqrt(3.0 * CH)
NT = N // 128     # 16 n-tiles
P = 128


def ts(i, s):
    return slice(i * s, (i + 1) * s)


def build_nc(nrep=1):
    nc = bacc.Bacc("TRN2", target_bir_lowering=False, debug=False)

    xs = {
        t: nc.dram_tensor(f"x{t}", [EMB, 3, N], BF16, kind="ExternalInput").ap()
        for t in ("q", "k", "z")
    }
    ws = {
        t: nc.dram_tensor(f"w{t}", [EMB, P], BF16, kind="ExternalInput").ap()
        for t in ("q", "k", "z")
    }
    wo = nc.dram_tensor("wo", [P, EMB], BF16, kind="ExternalInput").ap()
    us = {
        t: nc.dram_tensor(f"u{t}", [P, 3], FP32, kind="ExternalInput").ap()
        for t in ("q", "k", "z")
    }
    ident = nc.dram_tensor("ident", [P, P], BF16, kind="ExternalInput").ap()
    y = nc.dram_tensor("y", [EMB, 3, N], BF16, kind="ExternalOutput").ap()

    xr = {t: x.rearrange("(c p) d t -> p c d t", p=P) for t, x in xs.items()}
    wr = {t: w.rearrange("(c p) e -> p c e", p=P) for t, w in ws.items()}
    yr = y.rearrange("(c p) d t -> p c d t", p=P)

    with tile.TileContext(nc) as tc:
        with ExitStack() as ctx:
            pool = lambda name, bufs, **kw: ctx.enter_context(
                tc.tile_pool(name=name, bufs=bufs, **kw)
            )
            consts = pool("consts", 1)
            xin_pool = pool("xin", 2)
            proj_pool = pool("proj", 2)
            qf_pool = pool("qf", 2)
            kf_pool = pool("kf", 2)
            zft_pool = pool("zft", 2)
            ex_pool = pool("ex", 2)
            avsb_pool = pool("avsb", 2)
            inv_pool = pool("inv", 2)
            outh_pool = pool("outh", 1)
            outall_pool = pool("outall", 1)
            y_pool = pool("ysb", 2)
            pst_pool = pool("pst", 2, space="PSUM")
            pav_pool = pool("pav", 1, space="PSUM")
            pzt_pool = pool("pzt", 1, space="PSUM")
            pinv_pool = pool("pinv", 1, space="PSUM")

            # constants
            w_sb = {}
            u_sb = {}
            for t in ("q", "k", "z"):
                w_sb[t] = consts.tile([P, 2, P], BF16, tag=f"w{t}", name=f"w{t}_sb")
                nc.sync.dma_start(out=w_sb[t], in_=wr[t])
                u_sb[t] = consts.tile([P, 3], FP32, tag=f"u{t}", name=f"u{t}_sb")
                nc.sync.dma_start(out=u_sb[t], in_=us[t])
            wo_sb = consts.tile([P, EMB], BF16, tag="wo")
            nc.sync.dma_start(out=wo_sb, in_=wo)
            # ident4: I32 stacked 4x down the partitions, so the transpose
            # rhs can sit at the same base partition as the z-proj slice
            ident4 = consts.tile([P, 32], BF16, tag="ident4")
            for hh in range(4):
                nc.sync.dma_start(out=ident4[ts(hh, 32), :], in_=ident[:32, :32])
            ones96 = consts.tile([1, 96], BF16, tag="ones96")
            nc.vector.memset(ones96, 1.0)

            state = {}  # per-rep tiles, filled by preamble(rep)

            def preamble(rep):
                """Loads + projections + fanouts + z transposes for `rep`,
                yielded as one PE/DMA-sized piece per next()."""
                st_r = {}
                state[rep] = st_r
                xin = {}
                for t in ("q", "k"):
                    xin[t] = xin_pool.tile(
                        [P, 2, 3, N], BF16, tag="xin", name=f"x{t}in"
                    )
                    nc.sync.dma_start(out=xin[t], in_=xr[t])
                yield
                qf = st_r["qf"] = qf_pool.tile([96, HL, N], BF16, tag="qf", name="qf")
                kf = st_r["kf"] = kf_pool.tile([96, HL, N], BF16, tag="kf", name="kf")
                projs = {}
                for t in ("q", "k", "z"):
                    proj = proj_pool.tile([P, 3, N], BF16, tag="proj", name=f"p{t}")
                    projs[t] = proj
                    for d in range(3):
                        for nt in range(2):
                            ps = pst_pool.tile(
                                [P, 1024], FP32, tag="pst", name="projps"
                            )
                            for hf in range(2):
                                for cc in range(2):
                                    nc.tensor.matmul(
                                        ps[:, ts(hf, 512)],
                                        lhsT=w_sb[t][:, cc, :],
                                        rhs=xin[t][:, cc, d, ts(2 * nt + hf, 512)],
                                        start=(cc == 0),
                                        stop=(cc == 1),
                                    )
                            nc.vector.tensor_scalar_add(
                                proj[:, d, ts(nt, 1024)],
                                ps,
                                u_sb[t][:, d : d + 1],
                            )
                            yield
                        if t in ("q", "k"):
                            for c in range(2):
                                nc.sync.dma_start(
                                    out=st_r[t + "f"][ts(d, 32), :, ts(c, 1024)],
                                    in_=proj[:, d, ts(c, 1024)],
                                )
                    if t == "q":
                        # defer xz so its xin slot (shared with xq) is free
                        xin["z"] = xin_pool.tile(
                            [P, 2, 3, N], BF16, tag="xin", name="xzin"
                        )
                        nc.sync.dma_start(out=xin["z"], in_=xr["z"])
                        yield
                zfts = st_r["zfts"] = zft_pool.tile(
                    [P, HL, NT, 98], BF16, tag="zfts", name="zfts"
                )
                nc.vector.memset(
                    zfts.rearrange("p h n c -> p (h n) c")[:, :, 96:97], 1.0
                )
                pz = projs["z"]
                for h in range(HL):
                    for g in range(4):
                        zt = pzt_pool.tile([P, 4, 98], BF16, tag="pzt", name="zt")
                        for jj in range(4):
                            nt = 4 * g + jj
                            for d in range(3):
                                nc.tensor.transpose(
                                    zt[:, jj, ts(d, 32)],
                                    pz[ts(h, 32), d, ts(nt, P)],
                                    ident4[ts(h, 32), :],
                                    tile_position=(32 * h, 0),
                                )
                        nc.vector.tensor_copy(
                            zfts[:, h, ts(g, 4), :96], zt[:, :, :96]
                        )
                        yield

            def final_proj_pieces(rep, mh):
                """Gather + partial Wo projection for one m-half."""
                st_r = state[rep]
                for d in range(3):
                    nc.sync.dma_start(
                        out=st_r["out_all"][:, d, ts(mh, 1024)],
                        in_=st_r["outh"][ts(d, 32), :, mh, :],
                    )
                yield
                for d in range(3):
                    for mt in range(2):
                        m0 = 1024 * mh + 512 * mt
                        ps = pst_pool.tile([P, 1024], FP32, tag="pst", name="yps")
                        for eo in range(2):
                            nc.tensor.matmul(
                                ps[:, ts(eo, 512)],
                                lhsT=wo_sb[:, ts(eo, P)],
                                rhs=st_r["out_all"][:, d, m0 : m0 + 512],
                                start=True,
                                stop=True,
                            )
                        yp = y_pool.tile([P, 2, 512], BF16, tag="ysb", name="yp")
                        nc.vector.tensor_copy(
                            yp, ps.rearrange("p (e m) -> p e m", e=2)
                        )
                        nc.sync.dma_start(out=yr[:, :, d, m0 : m0 + 512], in_=yp)
                        yield

            work = deque()

            def pump():
                while work:
                    try:
                        next(work[0])
                        return
                    except StopIteration:
                        work.popleft()

            def attention(rep):
                # safety: if the drip slots ran out, finish this rep's
                # preamble before consuming its tiles
                while "zfts" not in state.get(rep, {}) and work:
                    pump()
                st_r = state[rep]
                qf, kf, zfts = st_r["qf"], st_r["kf"], st_r["zfts"]
                st_r["out_all"] = outall_pool.tile([P, 3, N], BF16, tag="outall", name="out_all")
                st_r["outh"] = outh_pool.tile([96, HL, 2, 1024], BF16, tag="outh", name="outh")
                for mh in range(2):
                    for h in range(HL):
                        av = pav_pool.tile([97, 1024], FP32, tag="pav", name="av")

                        def av_accum(nt, ex, av=av, h=h):
                            for mc in range(2):
                                nc.tensor.matmul(
                                    av[:, ts(mc, 512)],
                                    lhsT=zfts[:, h, nt, :97],
                                    rhs=ex[:, ts(mc, 512)],
                                    start=(nt == 0),
                                    stop=(nt == NT - 1),
                                )

                        # AV trails scores by one tile: exp(nt)'s input is
                        # ready a full slot early, so drip-piece PE spikes
                        # eat into the cushion instead of stalling ACT
                        prev_ex = None
                        for nt in range(NT):
                            st = pst_pool.tile(
                                [P, 1024], FP32, tag="pst", name="st"
                            )
                            for mc in range(2):
                                nc.tensor.matmul(
                                    st[:, ts(mc, 512)],
                                    lhsT=kf[:, h, ts(nt, P)],
                                    rhs=qf[:, h, ts(2 * mh + mc, 512)],
                                    start=True,
                                    stop=True,
                                )
                            ex = ex_pool.tile([P, 1024], BF16, tag="ex", name="ex")
                            nc.scalar.activation(ex, st, AF.Exp, scale=float(SCALE))
                            if prev_ex is not None:
                                av_accum(nt - 1, prev_ex)
                            prev_ex = ex
                            if nt % 2 == 1:
                                pump()
                        av_accum(NT - 1, prev_ex)
                        # normalize: row 96 of av is the softmax denominator.
                        # recip reads the fp32 PSUM row; the bulk av evicts
                        # to bf16 (its 0.4% rounding averages out in Wo)
                        inv = inv_pool.tile([1, 1024], BF16, tag="inv", name="inv")
                        with nc.allow_low_precision(reason="softmax inv fp16"):
                            nc.vector.reciprocal(inv, av[96:97, :])
                        av_sb = avsb_pool.tile(
                            [96, 1024], BF16, tag="avsb", name="av_sb"
                        )
                        nc.vector.tensor_copy(av_sb, av[0:96, :])
                        for mc in range(2):
                            invb = pinv_pool.tile(
                                [96, 512], FP32, tag="pinv", name="invb"
                            )
                            nc.tensor.matmul(
                                invb,
                                lhsT=ones96,
                                rhs=inv[:, ts(mc, 512)],
                                start=True,
                                stop=True,
                            )
                            nc.vector.tensor_tensor(
                                st_r["outh"][:, h, mh, ts(mc, 512)],
                                av_sb[0:96, ts(mc, 512)],
                                invb,
                                ALU.mult,
                            )
                    work.append(final_proj_pieces(rep, mh))

            for piece in preamble(0):  # first rep: standalone preamble
                pass
            for rep in range(nrep):
                if rep + 1 < nrep:
                    work.append(preamble(rep + 1))
                attention(rep)
            while work:  # drain the last rep's final projection
                pump()
                if not work:
                    break

    nc.compile()
    return nc


_NC_CACHE = {}


def get_nc():
    if "nc" not in _NC_CACHE:
        _NC_CACHE["nc"] = build_nc()
    return _NC_CACHE["nc"]


def _perm_cols(w):
    # [256, 128] -> columns reordered ch-major, head-minor
    return np.ascontiguousarray(
        w.reshape(EMB, HL, CH).transpose(0, 2, 1).reshape(EMB, P)
    )


def _perm_rows(a):
    # [128, ...] -> rows reordered ch-major, head-minor
    s = a.shape
    return np.ascontiguousarray(
        a.reshape(HL, CH, *s[1:]).transpose(1, 0, *range(2, 1 + len(s))).reshape(s)
    )


def make_in_maps(Q, K, Z, Wq_w, Wq_b, Wk_w, Wk_b, Wz_w, Wz_b, Wo_w, Wo_b):
    bf16 = mybir.dt.np(BF16)

    def u_of(b):
        b = np.asarray(b, np.float32)
        return (EPS * b / np.linalg.norm(b, axis=1, keepdims=True)).astype(np.float32)

    uq, uk, uz = u_of(Wq_b), u_of(Wk_b), u_of(Wz_b)
    Qb = np.asarray(Q).astype(bf16)
    Kb = np.asarray(K).astype(bf16)
    Zb = np.asarray(Z).astype(bf16)
    Wqb = np.asarray(Wq_w, np.float32)
    Wkb = np.asarray(Wk_w, np.float32)
    Wzb = np.asarray(Wz_w, np.float32)
    Wob = np.asarray(Wo_w, np.float32)
    ident = np.eye(P, dtype=np.float32).astype(bf16)

    in_maps = []
    for core in range(8):
        b, g = core // 2, core % 2
        cols = slice(P * g, P * (g + 1))
        in_maps.append(
            {
                "xq": np.ascontiguousarray(Qb[b]),
                "xk": np.ascontiguousarray(Kb[b]),
                "xz": np.ascontiguousarray(Zb[b]),
                "wq": _perm_cols(Wqb[:, cols]).astype(bf16),
                "wk": _perm_cols(Wkb[:, cols]).astype(bf16),
                "wz": np.ascontiguousarray(Wzb[:, cols]).astype(bf16),
                "wo": _perm_rows(np.ascontiguousarray(Wob[cols, :])).astype(bf16),
                "uq": _perm_rows(uq[cols]),
                "uk": _perm_rows(uk[cols]),
                "uz": np.ascontiguousarray(uz[cols]),
                "ident": ident,
            }
        )
    return in_maps


def assemble(results, Wo_b=None):
    out = np.empty((B, EMB, 3, N), dtype=np.float32)
    for b in range(B):
        out[b] = results[2 * b]["y"].astype(np.float32) + results[2 * b + 1][
            "y"
        ].astype(np.float32)
    if Wo_b is not None:
        bo = np.asarray(Wo_b, np.float32)
        uo = EPS * bo / np.linalg.norm(bo, axis=1, keepdims=True)
        out += uo[None, :, :, None]
    return out


def kernel(**inputs):
    nc = get_nc()
    in_maps = make_in_maps(**inputs)
    res = run_bass_kernel_spmd(nc, in_maps, list(range(8)))
    return assemble(res.results, Wo_b=inputs["Wo_b"])


if __name__ == "__main__":
    nc = build_nc()
    print("built ok")


# revision 6
# speedup vs baseline: 3.0696x; 1.0859x over previous
"""Trainium2 Bass kernel for vector-neuron multi-head attention, v2.2.

Sharding: 8 cores = 4 batches x 2 head-groups (tensor parallel). Each core
projects q/k/z for its 4 heads (128 of 256 output channels) over the full
M=N=2048 tokens, runs attention for those heads, and computes a PARTIAL
final Wo projection (contraction over its 128 channels only). The host
sums the two partials per batch and adds the Wo bias.

Layout tricks:
  - wq/wk columns are host-PERMUTED (ch-major, head-minor) so the per-head
    (d,ch)-packed qf/kf tiles are natural-partition-order fanout DMAs from
    the projection output ([128, chunk] -> [32, 4h, chunk]).
  - wz keeps the original head-contiguous layout; the AV operand zfts
    [n, (d*32+ch)+ones] is built by PE transposes DIRECTLY from the z
    projection (32-row tiles at base partition 32h via tile_position).
  - Wo rows get the q/k permutation so the reverse gather DMA feeds the
    final matmul directly. y partials are written bf16 (summed on host).

Scheduling: every engine queue is in-order, so cross-step overlap must be
EMITTED interleaved. All non-attention work (next rep's input loads,
projections, fanouts, transposes; this rep's per-m-half final projection)
is chopped into generator pieces and dripped one per odd n-tile slot of
the attention blocks (8 blocks x 16 slots/rep). ACT then runs its 128
softmax exps (~133us/rep) nearly back-to-back, with PE (~136us/rep)
co-bound. ~30 DMAs/rep.
"""

from collections import deque
from contextlib import ExitStack

import numpy as np

import concourse.bacc as bacc
import concourse.bass as bass
import concourse.tile as tile
from concourse import mybir
from concourse.bass_utils import run_bass_kernel_spmd

FP32 = mybir.dt.float32
BF16 = mybir.dt.float16  # fp16: 10 mantissa bits, same PE speed as bf16
AF = mybir.ActivationFunctionType
ALU = mybir.AluOpType

EMB = 256
HEADS = 8
EPS = 1e-6
B = 4
N = 2048          # tokens (M = N here)
HL = 4            # heads per core
CH = 32           # channels per head
SCALE = 1.0 / np.sqrt(3.0 * CH)
NT = N // 128     # 16 n-tiles
P = 128


def ts(i, s):
    return slice(i * s, (i + 1) * s)


def build_nc(nrep=1):
    nc = bacc.Bacc("TRN2", target_bir_lowering=False, debug=False)

    xs = {
        t: nc.dram_tensor(f"x{t}", [EMB, 3, N], BF16, kind="ExternalInput").ap()
        for t in ("q", "k", "z")
    }
    ws = {
        t: nc.dram_tensor(f"w{t}", [EMB, P], BF16, kind="ExternalInput").ap()
        for t in ("q", "k", "z")
    }
    wo = nc.dram_tensor("wo", [P, EMB], BF16, kind="ExternalInput").ap()
    us = {
        t: nc.dram_tensor(f"u{t}", [P, 3], FP32, kind="ExternalInput").ap()
        for t in ("q", "k", "z")
    }
    ident = nc.dram_tensor("ident", [P, P], BF16, kind="ExternalInput").ap()
    y = nc.dram_tensor("y", [EMB, 3, N], BF16, kind="ExternalOutput").ap()

    xr = {t: x.rearrange("(c p) d t -> p c d t", p=P) for t, x in xs.items()}
    wr = {t: w.rearrange("(c p) e -> p c e", p=P) for t, w in ws.items()}
    yr = y.rearrange("(c p) d t -> p c d t", p=P)

    with tile.TileContext(nc) as tc:
        with ExitStack() as ctx:
            pool = lambda name, bufs, **kw: ctx.enter_context(
                tc.tile_pool(name=name, bufs=bufs, **kw)
            )
            consts = pool("consts", 1)
            xin_pool = pool("xin", 2)
            proj_pool = pool("proj", 2)
            qf_pool = pool("qf", 2)
            kf_pool = pool("kf", 2)
            zft_pool = pool("zft", 2)
            ex_pool = pool("ex", 3)
            avsb_pool = pool("avsb", 2)
            inv_pool = pool("inv", 1)
            outh_pool = pool("outh", 1)
            outall_pool = pool("outall", 1)
            y_pool = pool("ysb", 2)
            pst_pool = pool("pst", 2, space="PSUM")
            pav_pool = pool("pav", 1, space="PSUM")
            pzt_pool = pool("pzt", 1, space="PSUM")
            pinv_pool = pool("pinv", 1, space="PSUM")

            # constants
            w_sb = {}
            u_sb = {}
            for t in ("q", "k", "z"):
                w_sb[t] = consts.tile([P, 2, P], BF16, tag=f"w{t}", name=f"w{t}_sb")
                nc.sync.dma_start(out=w_sb[t], in_=wr[t])
                u_sb[t] = consts.tile([P, 3], FP32, tag=f"u{t}", name=f"u{t}_sb")
                nc.sync.dma_start(out=u_sb[t], in_=us[t])
            wo_sb = consts.tile([P, EMB], BF16, tag="wo")
            nc.sync.dma_start(out=wo_sb, in_=wo)
            # ident4: I32 stacked 4x down the partitions, so the transpose
            # rhs can sit at the same base partition as the z-proj slice
            ident4 = consts.tile([P, 32], BF16, tag="ident4")
            for hh in range(4):
                nc.sync.dma_start(out=ident4[ts(hh, 32), :], in_=ident[:32, :32])
            ones96 = consts.tile([1, 96], BF16, tag="ones96")
            nc.vector.memset(ones96, 1.0)

            state = {}  # per-rep tiles, filled by preamble(rep)

            def preamble(rep):
                """Loads + projections + fanouts + z transposes for `rep`,
                yielded as one PE/DMA-sized piece per next()."""
                st_r = {}
                state[rep] = st_r
                xin = {}
                for t in ("q", "k"):
                    xin[t] = xin_pool.tile(
                        [P, 2, 3, N], BF16, tag="xin", name=f"x{t}in"
                    )
                    nc.sync.dma_start(out=xin[t], in_=xr[t])
                yield
                qf = st_r["qf"] = qf_pool.tile([96, HL, N], BF16, tag="qf", name="qf")
                kf = st_r["kf"] = kf_pool.tile([96, HL, N], BF16, tag="kf", name="kf")
                projs = {}
                for t in ("q", "k", "z"):
                    proj = proj_pool.tile([P, 3, N], BF16, tag="proj", name=f"p{t}")
                    projs[t] = proj
                    for d in range(3):
                        for nt in range(2):
                            ps = pst_pool.tile(
                                [P, 1024], FP32, tag="pst", name="projps"
                            )
                            for hf in range(2):
                                for cc in range(2):
                                    nc.tensor.matmul(
                                        ps[:, ts(hf, 512)],
                                        lhsT=w_sb[t][:, cc, :],
                                        rhs=xin[t][:, cc, d, ts(2 * nt + hf, 512)],
                                        start=(cc == 0),
                                        stop=(cc == 1),
                                    )
                            nc.vector.tensor_scalar_add(
                                proj[:, d, ts(nt, 1024)],
                                ps,
                                u_sb[t][:, d : d + 1],
                            )
                            yield
                        if t in ("q", "k"):
                            for c in range(2):
                                nc.sync.dma_start(
                                    out=st_r[t + "f"][ts(d, 32), :, ts(c, 1024)],
                                    in_=proj[:, d, ts(c, 1024)],
                                )
                    if t == "q":
                        # defer xz so its xin slot (shared with xq) is free
                        xin["z"] = xin_pool.tile(
                            [P, 2, 3, N], BF16, tag="xin", name="xzin"
                        )
                        nc.sync.dma_start(out=xin["z"], in_=xr["z"])
                        yield
                zfts = st_r["zfts"] = zft_pool.tile(
                    [P, HL, NT, 98], BF16, tag="zfts", name="zfts"
                )
                nc.vector.memset(
                    zfts.rearrange("p h n c -> p (h n) c")[:, :, 96:97], 1.0
                )
                pz = projs["z"]
                for h in range(HL):
                    for g in range(4):
                        zt = pzt_pool.tile([P, 4, 98], BF16, tag="pzt", name="zt")
                        for jj in range(4):
                            nt = 4 * g + jj
                            for d in range(3):
                                nc.tensor.transpose(
                                    zt[:, jj, ts(d, 32)],
                                    pz[ts(h, 32), d, ts(nt, P)],
                                    ident4[ts(h, 32), :],
                                    tile_position=(32 * h, 0),
                                )
                        nc.vector.tensor_copy(
                            zfts[:, h, ts(g, 4), :96], zt[:, :, :96]
                        )
                        yield

            def final_proj_pieces(rep, mh):
                """Gather + partial Wo projection for one m-half."""
                st_r = state[rep]
                for d in range(3):
                    nc.sync.dma_start(
                        out=st_r["out_all"][:, d, ts(mh, 1024)],
                        in_=st_r["outh"][ts(d, 32), :, mh, :],
                    )
                yield
                yield
                yield
                for d in range(3):
                    for mt in range(2):
                        m0 = 1024 * mh + 512 * mt
                        ps = pst_pool.tile([P, 1024], FP32, tag="pst", name="yps")
                        for eo in range(2):
                            nc.tensor.matmul(
                                ps[:, ts(eo, 512)],
                                lhsT=wo_sb[:, ts(eo, P)],
                                rhs=st_r["out_all"][:, d, m0 : m0 + 512],
                                start=True,
                                stop=True,
                            )
                        yp = y_pool.tile([P, 2, 512], BF16, tag="ysb", name="yp")
                        nc.vector.tensor_copy(
                            yp, ps.rearrange("p (e m) -> p e m", e=2)
                        )
                        nc.sync.dma_start(out=yr[:, :, d, m0 : m0 + 512], in_=yp)
                        yield

            work = deque()

            def pump():
                while work:
                    try:
                        next(work[0])
                        return
                    except StopIteration:
                        work.popleft()

            def attention(rep):
                # safety: if the drip slots ran out, finish this rep's
                # preamble before consuming its tiles
                while "zfts" not in state.get(rep, {}) and work:
                    pump()
                st_r = state[rep]
                qf, kf, zfts = st_r["qf"], st_r["kf"], st_r["zfts"]
                st_r["out_all"] = outall_pool.tile([P, 3, N], BF16, tag="outall", name="out_all")
                st_r["outh"] = outh_pool.tile([96, HL, 2, 1024], BF16, tag="outh", name="outh")
                for mh in range(2):
                    for h in range(HL):
                        av = pav_pool.tile([97, 1024], FP32, tag="pav", name="av")

                        def av_accum(nt, ex, av=av, h=h):
                            for mc in range(2):
                                nc.tensor.matmul(
                                    av[:, ts(mc, 512)],
                                    lhsT=zfts[:, h, nt, :97],
                                    rhs=ex[:, ts(mc, 512)],
                                    start=(nt == 0),
                                    stop=(nt == NT - 1),
                                )

                        # AV trails scores by one tile: exp(nt)'s input is
                        # ready a full slot early, so drip-piece PE spikes
                        # eat into the cushion instead of stalling ACT
                        prev_ex = None
                        for nt in range(NT):
                            st = pst_pool.tile(
                                [P, 1024], FP32, tag="pst", name="st"
                            )
                            for mc in range(2):
                                nc.tensor.matmul(
                                    st[:, ts(mc, 512)],
                                    lhsT=kf[:, h, ts(nt, P)],
                                    rhs=qf[:, h, ts(2 * mh + mc, 512)],
                                    start=True,
                                    stop=True,
                                )
                            ex = ex_pool.tile([P, 1024], BF16, tag="ex", name="ex")
                            nc.scalar.activation(ex, st, AF.Exp, scale=float(SCALE))
                            if prev_ex is not None:
                                av_accum(nt - 1, prev_ex)
                            prev_ex = ex
                            if nt % 2 == 1:
                                pump()
                        av_accum(NT - 1, prev_ex)
                        # normalize: row 96 of av is the softmax denominator.
                        # recip reads the fp32 PSUM row; the bulk av evicts
                        # to bf16 (its 0.4% rounding averages out in Wo)
                        inv = inv_pool.tile([1, 1024], BF16, tag="inv", name="inv")
                        with nc.allow_low_precision(reason="softmax inv fp16"):
                            nc.vector.reciprocal(inv, av[96:97, :])
                        av_sb = avsb_pool.tile(
                            [96, 1024], BF16, tag="avsb", name="av_sb"
                        )
                        nc.vector.tensor_copy(av_sb, av[0:96, :])
                        for mc in range(2):
                            invb = pinv_pool.tile(
                                [96, 512], FP32, tag="pinv", name="invb"
                            )
                            nc.tensor.matmul(
                                invb,
                                lhsT=ones96,
                                rhs=inv[:, ts(mc, 512)],
                                start=True,
                                stop=True,
                            )
                            nc.vector.tensor_tensor(
                                st_r["outh"][:, h, mh, ts(mc, 512)],
                                av_sb[0:96, ts(mc, 512)],
                                invb,
                                ALU.mult,
                            )
                    work.append(final_proj_pieces(rep, mh))

            for piece in preamble(0):  # first rep: standalone preamble
                pass
            for rep in range(nrep):
                if rep + 1 < nrep:
                    work.append(preamble(rep + 1))
                attention(rep)
            while work:  # drain the last rep's final projection
                pump()
                if not work:
                    break

    nc.compile()
    return nc


_NC_CACHE = {}


def get_nc():
    if "nc" not in _NC_CACHE:
        _NC_CACHE["nc"] = build_nc()
    return _NC_CACHE["nc"]


def _perm_cols(w):
    # [256, 128] -> columns reordered ch-major, head-minor
    return np.ascontiguousarray(
        w.reshape(EMB, HL, CH).transpose(0, 2, 1).reshape(EMB, P)
    )


def _perm_rows(a):
    # [128, ...] -> rows reordered ch-major, head-minor
    s = a.shape
    return np.ascontiguousarray(
        a.reshape(HL, CH, *s[1:]).transpose(1, 0, *range(2, 1 + len(s))).reshape(s)
    )


def make_in_maps(Q, K, Z, Wq_w, Wq_b, Wk_w, Wk_b, Wz_w, Wz_b, Wo_w, Wo_b):
    bf16 = mybir.dt.np(BF16)

    def u_of(b):
        b = np.asarray(b, np.float32)
        return (EPS * b / np.linalg.norm(b, axis=1, keepdims=True)).astype(np.float32)

    uq, uk, uz = u_of(Wq_b), u_of(Wk_b), u_of(Wz_b)
    Qb = np.asarray(Q).astype(bf16)
    Kb = np.asarray(K).astype(bf16)
    Zb = np.asarray(Z).astype(bf16)
    Wqb = np.asarray(Wq_w, np.float32)
    Wkb = np.asarray(Wk_w, np.float32)
    Wzb = np.asarray(Wz_w, np.float32)
    Wob = np.asarray(Wo_w, np.float32)
    ident = np.eye(P, dtype=np.float32).astype(bf16)

    in_maps = []
    for core in range(8):
        b, g = core // 2, core % 2
        cols = slice(P * g, P * (g + 1))
        in_maps.append(
            {
                "xq": np.ascontiguousarray(Qb[b]),
                "xk": np.ascontiguousarray(Kb[b]),
                "xz": np.ascontiguousarray(Zb[b]),
                "wq": _perm_cols(Wqb[:, cols]).astype(bf16),
                "wk": _perm_cols(Wkb[:, cols]).astype(bf16),
                "wz": np.ascontiguousarray(Wzb[:, cols]).astype(bf16),
                "wo": _perm_rows(np.ascontiguousarray(Wob[cols, :])).astype(bf16),
                "uq": _perm_rows(uq[cols]),
                "uk": _perm_rows(uk[cols]),
                "uz": np.ascontiguousarray(uz[cols]),
                "ident": ident,
            }
        )
    return in_maps


def assemble(results, Wo_b=None):
    out = np.empty((B, EMB, 3, N), dtype=np.float32)
    for b in range(B):
        out[b] = results[2 * b]["y"].astype(np.float32) + results[2 * b + 1][
            "y"
        ].astype(np.float32)
    if Wo_b is not None:
        bo = np.asarray(Wo_b, np.float32)
        uo = EPS * bo / np.linalg.norm(bo, axis=1, keepdims=True)
        out += uo[None, :, :, None]
    return out


def kernel(**inputs):
    nc = get_nc()
    in_maps = make_in_maps(**inputs)
    res = run_bass_kernel_spmd(nc, in_maps, list(range(8)))
    return assemble(res.results, Wo_b=inputs["Wo_b"])


if __name__ == "__main__":
    nc = build_nc()
    print("built ok")
